# revision 12
# baseline (speedup 1.0000x reference)
"""Trainium2 Bass kernel for the attention-pooling module (v3).

Reference math (B=32, N=2048, D=512, K=256):
    vIp   = vI @ Wi                                   [B,N,K]
    vQp   = vQ @ Wq + bq                              [B,K]
    ha    = leaky_relu(vIp + vQp[:,None,:], 0.01)     [B,N,K]
    scores= ha @ Wp[:,0] + bp                         [B,N]   (bp cancels in softmax)
    pi    = softmax(scores, -1)                       [B,N]
    out   = einsum("bn,bnk->bk", pi, vIp) + vQp       [B,K]

Device (per core, 4 batches, data-parallel over B) computes only what needs
the bulk tensor:
    scores path: vpT = (16*Wi)^T @ vIT   (fp8 DoubleRow matmuls, K on partitions)
                 ha  = ACT Lrelu(vpT/16 + vqpT)  -> fp8
                 scores = (8*Wp)^T @ ha  (PE)  -> exp on [128,16] layout -> e fp8
    attention:   u = e @ vI  (fp8 DoubleRow matmuls vs the natural-layout copy)
                 Z = sum(e)  via the exp's accum_out
Host computes the precision-critical small math exactly in fp32:
    vQp = vQ@Wq + bq,  att = (u @ Wi)/Z,  out = att + vQp.
The device-side wq/vq/bq only shape the softmax weights, so they ride bf16.

Schedule notes:
  - 9 warm-up matmuls on a zeroed tile spin the PE HAM clock-gate up to
    2.4 GHz during the DMA preamble (otherwise the whole kernel runs at
    1.2 GHz in bursts: 39 us of throttled PE time in the v2 trace).
  - DRAM layouts keep each partition's data contiguous (128-descriptor DMAs;
    ~0.6 us HWDGE issue each instead of 1.5 us for 512-descriptor patterns).
  - Per-batch loads are interleaved vit(b+1) before vnat(b) so the PE never
    waits on the next scores tile; u(b) matmuls slot between scores(b+1)
    and scores(b+2).
"""

import os
import sys

sys.path.insert(0, "/opt/trn_rl_repo")

import numpy as np
import ml_dtypes

from concourse import bass, bacc, tile, mybir
from concourse.bass_utils import run_bass_kernel_spmd

dt = mybir.dt
F32, BF16, FP8 = dt.float32, dt.bfloat16, dt.float8e4
AF = mybir.ActivationFunctionType
ALU = mybir.AluOpType

B, N, D, K = 32, 2048, 512, 256
NCORES = 8
BLOC = B // NCORES           # 4 batches per core
SUP = 512                    # scores supertile (PSUM-bank limited)
DC = D // 128                # 4
KC = K // 128                # 2
NEG = 0.01
NWARM = 9                    # ~3.8 us of warm-up matmuls


def build_nc():
    nc = bacc.Bacc("TRN2", target_bir_lowering=False, debug=False)

    # vit: d-on-partitions layout; [b, p, sp, cc, i, n1024], d = cc*256+i*128+p,
    # n = sp*1024 + n1024. Each partition row is 8 KiB contiguous in DRAM.
    vit_d = nc.dram_tensor("vit", [BLOC, 128, 2, 2, 2, 1024], FP8, kind="ExternalInput")
    # vnat: n-on-partitions layout; [b, pn, t, d], n = t*128 + pn.
    vnat_d = nc.dram_tensor("vnat", [BLOC, 128, 16, D], FP8, kind="ExternalInput")
    wi8_d = nc.dram_tensor("wi8", [128, 2, 2, K], FP8, kind="ExternalInput")
    # pk16: wq(1024) | vqt(16) | bq(2) | wp_dr(2x16, wp*8 in col j=0) | id16(16)
    pk16_d = nc.dram_tensor("pk16", [128, 1090], BF16, kind="ExternalInput")
    u_d = nc.dram_tensor("u", [1, BLOC, D], F32, kind="ExternalOutput")
    zp_d = nc.dram_tensor("zp", [128, BLOC], F32, kind="ExternalOutput")

    with tile.TileContext(nc) as tc:
        with (
            tc.tile_pool(name="const", bufs=1) as cpool,
            tc.tile_pool(name="stream", bufs=4) as spool,
            tc.tile_pool(name="work", bufs=3) as wpool,
            tc.tile_pool(name="pmm", bufs=3, space=bass.MemorySpace.PSUM) as pmm,
            tc.tile_pool(name="psm", bufs=2, space=bass.MemorySpace.PSUM) as psm,
        ):
            pk16_sb = cpool.tile([128, 1090], BF16, tag="pk16")
            wi8_sb = cpool.tile([128, 2, 2, K], FP8, tag="wi8")
            junk = cpool.tile([128, SUP], FP8, tag="junk")
            vqpt_sb = cpool.tile([128, KC, BLOC], F32, tag="vqpt")
            wp8 = cpool.tile([128, 2, 16], FP8, tag="wp8")
            u_sb = cpool.tile([1, BLOC, D], F32, tag="usb")
            zp_sb = cpool.tile([128, BLOC], F32, tag="zpsb")

            vit_tiles = [
                spool.tile([128, 2, 2, 2, 1024], FP8, tag="vit", name=f"vit{b}")
                for b in range(BLOC)
            ]
            vnat_tiles = [
                spool.tile([128, 16, D], FP8, tag="vnat", name=f"vnat{b}")
                for b in range(BLOC)
            ]

            # ---- input DMAs, just-in-time order --------------------------
            nc.sync.dma_start(out=pk16_sb[:], in_=pk16_d[:])
            nc.sync.dma_start(out=vit_tiles[0][:, 0], in_=vit_d[0][:, 0])
            nc.sync.dma_start(out=wi8_sb[:], in_=wi8_d[:])
            nc.sync.dma_start(out=vit_tiles[0][:, 1], in_=vit_d[0][:, 1])
            nc.sync.dma_start(out=vit_tiles[1][:], in_=vit_d[1])
            nc.sync.dma_start(out=vnat_tiles[0][:], in_=vnat_d[0])
            nc.sync.dma_start(out=vit_tiles[2][:], in_=vit_d[2])
            nc.sync.dma_start(out=vnat_tiles[1][:], in_=vnat_d[1])
            nc.sync.dma_start(out=vit_tiles[3][:], in_=vit_d[3])
            nc.sync.dma_start(out=vnat_tiles[2][:], in_=vnat_d[2])
            nc.sync.dma_start(out=vnat_tiles[3][:], in_=vnat_d[3])

            # ---- PE warm-up: HAM un-throttles after ~3.4 us of activity --
            nc.gpsimd.memset(junk[:], 0)
            for w in range(NWARM):
                wps = psm.tile([128, SUP], F32, tag="small", name=f"warm{w}")
                nc.tensor.matmul(
                    wps[:], junk[:, 0:128], junk[:], start=True, stop=True
                )
            # preload the ACT tables (Lrelu/Exp/Copy) while the DMAs stream;
            # a lazy mid-kernel ACT_TABLE_LOAD costs 1.28us on the e-chain
            actw = wpool.tile([128, 16], BF16, tag="actw")
            nc.scalar.activation(actw[:], junk[:, 0:16], AF.Prelu, alpha=NEG)
            nc.scalar.activation(actw[:], junk[:, 0:16], AF.Exp)
            nc.scalar.copy(actw[:], junk[:, 0:16])  # Copy shares the exp table set

            # ---- vqpT bias columns: vqp^T[k,b] = sum_d Wq[d,k] vQ[b,d] + bq
            wq16 = pk16_sb[:, 0:1024].rearrange("p (c k) -> p c k", c=DC)
            vqt16 = pk16_sb[:, 1024:1040].rearrange("p (c b) -> p c b", c=DC)
            bq16 = pk16_sb[:, 1040:1042]
            wp16 = pk16_sb[:, 1042:1074].rearrange("p (i j) -> p i j", i=2)
            id16 = pk16_sb[:, 1074:1090]
            nc.vector.tensor_copy(wp8[:], wp16[:])
            bq32 = cpool.tile([128, KC], F32, tag="bq32")
            nc.vector.tensor_copy(bq32[:], bq16[:])
            for kc in range(KC):
                vqpt_ps = psm.tile([128, BLOC], F32, tag="small", name=f"vqps{kc}")
                for c in range(DC):
                    nc.tensor.matmul(
                        vqpt_ps[:],
                        wq16[:, c, kc * 128 : (kc + 1) * 128],
                        vqt16[:, c, :],
                        start=(c == 0),
                        stop=(c == DC - 1),
                    )
                nc.vector.tensor_scalar(
                    vqpt_sb[:, kc, :], vqpt_ps[:], bq32[:, kc : kc + 1], None, ALU.add
                )

            scrows = [None] * BLOC

            def phase_scores(b):
                vit = vit_tiles[b]
                scrow = wpool.tile([1, N], BF16, tag="scrow", name=f"scrow{b}")
                scrows[b] = scrow
                for sp in range(2):
                    scps = [
                        psm.tile([1, SUP], F32, tag="small", name=f"scp{b}_{sp}_{h}")
                        for h in range(2)
                    ]
                    ha = wpool.tile([128, KC, 1024], FP8, tag="ha", name=f"ha{b}{sp}")
                    for kc in range(KC):
                        vp = pmm.tile([128, 1024], F32, tag="vp", name=f"vp{b}{sp}{kc}")
                        for cc in range(2):
                            for h in range(2):
                                nc.tensor.matmul(
                                    vp[:, h * SUP : (h + 1) * SUP],
                                    wi8_sb[:, cc, :, kc * 128 : (kc + 1) * 128],
                                    vit[:, sp, cc, :, h * SUP : (h + 1) * SUP],
                                    perf_mode=mybir.MatmulPerfMode.DoubleRow,
                                    start=(cc == 0),
                                    stop=(cc == 1),
                                )
                        # Wi is host-scaled x16 into fp8 normal range; ACT
                        # de-scales for free: ha = lrelu(vp/16 + vqp)
                        nc.scalar.activation(
                            ha[:, kc, :], vp[:], AF.Prelu,
                            bias=vqpt_sb[:, kc, b : b + 1], scale=1.0 / 16, alpha=NEG,
                        )
                    for h in range(2):
                        nc.tensor.matmul(
                            scps[h][:], wp8[:, :, 0:1],
                            ha[:, :, h * SUP : (h + 1) * SUP],
                            perf_mode=mybir.MatmulPerfMode.DoubleRow,
                            start=True, stop=True,
                        )
                    for h in range(2):
                        n0 = sp * 1024 + h * SUP
                        nc.vector.tensor_copy(scrow[0:1, n0 : n0 + SUP], scps[h][:])

            def phase_attn(b):
                vnat, scrow = vnat_tiles[b], scrows[b]
                # scores -> [16,128] (small SBUF-SBUF gather) -> PE transpose
                # -> PSUM [128,16], exp reads PSUM. (The xbar dma transpose
                # serializes against every in-flight DMA and stalled the
                # e-chain behind the whole input stream.)
                s16 = wpool.tile([16, 128], BF16, tag="s16", name=f"s16_{b}")
                nc.sync.dma_start(
                    out=s16[:], in_=scrow[0:1, :].rearrange("o (t p) -> o t p", p=128)
                )
                s_ps = psm.tile([128, 16], BF16, tag="small", name=f"sps{b}")
                nc.tensor.transpose(s_ps[:], s16[:], id16[0:16, :])

                # [128, 2, 16]: pair partner at +16B so the DoubleRow
                # lhsT AP satisfies the 16B-step ISA constraint.
                # Wp is host-scaled x8 (fp8 range); exp de-scales for free.
                e_col = wpool.tile([128, 2, 16], FP8, tag="ecol", name=f"ecol{b}")
                nc.scalar.activation(
                    e_col[:].rearrange("p i j -> p j i")[:, 0:8, :],
                    s_ps[:].rearrange("p (j i) -> p j i", i=2),
                    AF.Exp, scale=1.0 / 8, accum_out=zp_sb[:, b : b + 1],
                )

                # u = e @ vI on the PE: 8 accumulating fp8 DoubleRow matmuls
                ups = psm.tile([1, D], F32, tag="small", name=f"ups{b}")
                NT = N // 128
                for t in range(0, NT, 2):
                    nc.tensor.matmul(
                        ups[:],
                        e_col[:, :, t // 2 : t // 2 + 1],  # pair stride 16B
                        vnat[:, t : t + 2, :],
                        perf_mode=mybir.MatmulPerfMode.DoubleRow,
                        start=(t == 0),
                        stop=(t == NT - 2),
                    )
                nc.vector.tensor_copy(u_sb[:, b, :], ups[:])

            # software pipeline: scores(b) overlaps attention(b-1)
            for b in range(BLOC + 1):
                if b < BLOC:
                    phase_scores(b)
                if b >= 1:
                    phase_attn(b - 1)

            nc.sync.dma_start(out=u_d[:], in_=u_sb[:])
            nc.sync.dma_start(out=zp_d[:], in_=zp_sb[:])

    nc.compile()
    return nc


_NC = None


def _get_nc():
    global _NC
    if _NC is None:
        _NC = build_nc()
    return _NC


def kernel(vI, vQ, Wi, Wq, bq, Wp, bp, **_unused):
    vI = np.asarray(vI, dtype=np.float32)
    vQ = np.asarray(vQ, dtype=np.float32)
    Wi = np.asarray(Wi, dtype=np.float32)
    Wq = np.asarray(Wq, dtype=np.float32)
    bq = np.asarray(bq, dtype=np.float32)
    Wp = np.asarray(Wp, dtype=np.float32)
    # bp shifts every score equally -> cancels in softmax; ignored (and 0).

    bf = ml_dtypes.bfloat16
    f8 = ml_dtypes.float8_e4m3

    vi8 = vI.astype(f8)
    # vit: [B, p, sp, cc, i, n1024]; d = cc*256 + i*128 + p, n = sp*1024 + n'
    vit = np.ascontiguousarray(
        vi8.reshape(B, 2, 1024, 2, 2, 128).transpose(0, 5, 1, 3, 4, 2)
    )
    # vnat: [B, pn, t, d]; n = t*128 + pn
    vnat = np.ascontiguousarray(
        vi8.reshape(B, 16, 128, D).transpose(0, 2, 1, 3)
    )
    wi8 = np.ascontiguousarray(
        (Wi * 16.0).reshape(2, 2, 128, K).transpose(2, 0, 1, 3)
    ).astype(f8)                                                  # [128,cc,i,K]
    wq_h = Wq.reshape(DC, 128, K).transpose(1, 0, 2).reshape(128, DC * K)
    bq_h = bq.reshape(KC, 128).T                                  # [128,KC]
    wp_pad = np.zeros((128, 2, 16), np.float32)
    wp_pad[:, :, 0] = Wp[:, 0].reshape(KC, 128).T * 8.0
    id16 = np.zeros((128, 16), np.float32)
    id16[0:16, :] = np.eye(16, dtype=np.float32)

    def pk16_for(core):
        vqc = vQ[core * BLOC : (core + 1) * BLOC]                 # [BLOC, D]
        vqt = vqc.T.reshape(DC, 128, BLOC).transpose(1, 0, 2)     # [128,DC,BLOC]
        return np.ascontiguousarray(
            np.concatenate(
                [wq_h, vqt.reshape(128, DC * BLOC), bq_h,
                 wp_pad.reshape(128, 32), id16],
                axis=1,
            ).astype(bf)
        )

    in_maps = []
    for c in range(NCORES):
        in_maps.append(
            {
                "vit": vit[c * BLOC : (c + 1) * BLOC],
                "vnat": vnat[c * BLOC : (c + 1) * BLOC],
                "wi8": wi8,
                "pk16": pk16_for(c),
            }
        )

    nc = _get_nc()
    res = run_bass_kernel_spmd(
        nc, in_maps, list(range(NCORES)),
        trace=bool(int(os.environ.get("KERNEL_TRACE", "0"))),
        tmpdir=globals().get("TRACE_TMPDIR"),
    )
    kernel.last_results = res

    # host-side exact fp32 finish: out = (u @ Wi)/Z + vQp
    vQp = vQ @ Wq + bq
    out = np.empty((B, K), np.float32)
    for c in range(NCORES):
        u = np.asarray(res.results[c]["u"], np.float32)[0]        # [BLOC, D]
        zp = np.asarray(res.results[c]["zp"], np.float32)         # [128, BLOC]
        Z = zp.sum(axis=0)                                        # [BLOC]
        att = (u @ Wi) / Z[:, None]                               # [BLOC, K]
        out[c * BLOC : (c + 1) * BLOC] = att + vQp[c * BLOC : (c + 1) * BLOC]
    return out


# revision 15
# speedup vs baseline: 1.0678x; 1.0678x over previous
"""Trainium2 Bass kernel for the attention-pooling module (v3).

Reference math (B=32, N=2048, D=512, K=256):
    vIp   = vI @ Wi                                   [B,N,K]
    vQp   = vQ @ Wq + bq                              [B,K]
    ha    = leaky_relu(vIp + vQp[:,None,:], 0.01)     [B,N,K]
    scores= ha @ Wp[:,0] + bp                         [B,N]   (bp cancels in softmax)
    pi    = softmax(scores, -1)                       [B,N]
    out   = einsum("bn,bnk->bk", pi, vIp) + vQp       [B,K]

Device (per core, 4 batches, data-parallel over B) computes only what needs
the bulk tensor:
    scores path: vpT = (16*Wi)^T @ vIT   (fp8 DoubleRow matmuls, K on partitions)
                 ha  = ACT Lrelu(vpT/16 + vqpT)  -> fp8
                 scores = (8*Wp)^T @ ha  (PE)  -> exp on [128,16] layout -> e fp8
    attention:   u = e @ vI  (fp8 DoubleRow matmuls vs the natural-layout copy)
                 Z = sum(e)  via the exp's accum_out
Host computes the precision-critical small math exactly in fp32:
    vQp = vQ@Wq + bq,  att = (u @ Wi)/Z,  out = att + vQp.
The device-side wq/vq/bq only shape the softmax weights, so they ride bf16.

Schedule notes:
  - 9 warm-up matmuls on a zeroed tile spin the PE HAM clock-gate up to
    2.4 GHz during the DMA preamble (otherwise the whole kernel runs at
    1.2 GHz in bursts: 39 us of throttled PE time in the v2 trace).
  - DRAM layouts keep each partition's data contiguous (128-descriptor DMAs;
    ~0.6 us HWDGE issue each instead of 1.5 us for 512-descriptor patterns).
  - Per-batch loads are interleaved vit(b+1) before vnat(b) so the PE never
    waits on the next scores tile; u(b) matmuls slot between scores(b+1)
    and scores(b+2).
"""

import os
import sys

sys.path.insert(0, "/opt/trn_rl_repo")

import numpy as np
import ml_dtypes

from concourse import bass, bacc, tile, mybir
from concourse.bass_utils import run_bass_kernel_spmd

dt = mybir.dt
F32, BF16, FP8 = dt.float32, dt.bfloat16, dt.float8e4
AF = mybir.ActivationFunctionType
ALU = mybir.AluOpType

B, N, D, K = 32, 2048, 512, 256
NCORES = 8
BLOC = B // NCORES           # 4 batches per core
SUP = 512                    # scores supertile (PSUM-bank limited)
DC = D // 128                # 4
KC = K // 128                # 2
NEG = 0.01
NWARM = 6                    # ~2.6 us of warm-up matmuls


def build_nc():
    nc = bacc.Bacc("TRN2", target_bir_lowering=False, debug=False)

    # vit: d-on-partitions layout; [b, p, sp, cc, i, n1024], d = cc*256+i*128+p,
    # n = sp*1024 + n1024. Each partition row is 8 KiB contiguous in DRAM.
    vit_d = nc.dram_tensor("vit", [BLOC, 128, 2, 2, 2, 1024], FP8, kind="ExternalInput")
    # vnat: n-on-partitions layout; [b, pn, t, d], n = t*128 + pn.
    vnat_d = nc.dram_tensor("vnat", [BLOC, 128, 16, D], FP8, kind="ExternalInput")
    wi8_d = nc.dram_tensor("wi8", [128, 2, 2, K], FP8, kind="ExternalInput")
    # pk16: wq(1024) | vqt(16) | bq(2) | wp_dr(2x16, wp*8 in col j=0) | id16(16)
    pk16_d = nc.dram_tensor("pk16", [128, 1090], BF16, kind="ExternalInput")
    u_d = nc.dram_tensor("u", [1, BLOC, D], F32, kind="ExternalOutput")
    zp_d = nc.dram_tensor("zp", [128, BLOC], F32, kind="ExternalOutput")

    with tile.TileContext(nc) as tc:
        with (
            tc.tile_pool(name="const", bufs=1) as cpool,
            tc.tile_pool(name="stream", bufs=4) as spool,
            tc.tile_pool(name="work", bufs=3) as wpool,
            tc.tile_pool(name="pmm", bufs=3, space=bass.MemorySpace.PSUM) as pmm,
            tc.tile_pool(name="psm", bufs=2, space=bass.MemorySpace.PSUM) as psm,
        ):
            pk16_sb = cpool.tile([128, 1090], BF16, tag="pk16")
            wi8_sb = cpool.tile([128, 2, 2, K], FP8, tag="wi8")
            junk = cpool.tile([128, SUP], FP8, tag="junk")
            vqpt_sb = cpool.tile([128, KC, BLOC], F32, tag="vqpt")
            wp8 = cpool.tile([128, 2, 16], FP8, tag="wp8")
            u_sb = cpool.tile([1, BLOC, D], F32, tag="usb")
            zp_sb = cpool.tile([128, BLOC], F32, tag="zpsb")

            vit_tiles = [
                spool.tile([128, 2, 2, 2, 1024], FP8, tag="vit", name=f"vit{b}")
                for b in range(BLOC)
            ]
            vnat_tiles = [
                spool.tile([128, 16, D], FP8, tag="vnat", name=f"vnat{b}")
                for b in range(BLOC)
            ]

            # ---- input DMAs, just-in-time order --------------------------
            # vit0 lands in (sp, cc) quarters so the first matmul can start
            # as soon as ~0.7 MB (pk16+wi8+quarter) has streamed in.
            nc.sync.dma_start(out=pk16_sb[:], in_=pk16_d[:])
            nc.sync.dma_start(out=vit_tiles[0][:, 0, 0], in_=vit_d[0][:, 0, 0])
            nc.sync.dma_start(out=wi8_sb[:], in_=wi8_d[:])
            nc.sync.dma_start(out=vit_tiles[0][:, 0, 1], in_=vit_d[0][:, 0, 1])
            nc.sync.dma_start(out=vit_tiles[0][:, 1, 0], in_=vit_d[0][:, 1, 0])
            nc.sync.dma_start(out=vit_tiles[0][:, 1, 1], in_=vit_d[0][:, 1, 1])
            nc.sync.dma_start(out=vit_tiles[1][:], in_=vit_d[1])
            nc.sync.dma_start(out=vnat_tiles[0][:], in_=vnat_d[0])
            nc.sync.dma_start(out=vit_tiles[2][:], in_=vit_d[2])
            nc.sync.dma_start(out=vnat_tiles[1][:], in_=vnat_d[1])
            nc.sync.dma_start(out=vit_tiles[3][:], in_=vit_d[3])
            nc.sync.dma_start(out=vnat_tiles[2][:], in_=vnat_d[2])
            nc.sync.dma_start(out=vnat_tiles[3][:], in_=vnat_d[3])

            # ---- PE warm-up: HAM un-throttles after ~3.4 us of activity --
            nc.gpsimd.memset(junk[:], 0)
            for w in range(NWARM):
                wps = psm.tile([128, SUP], F32, tag="small", name=f"warm{w}")
                nc.tensor.matmul(
                    wps[:], junk[:, 0:128], junk[:], start=True, stop=True
                )
            # preload the ACT tables (Lrelu/Exp/Copy) while the DMAs stream;
            # a lazy mid-kernel ACT_TABLE_LOAD costs 1.28us on the e-chain
            actw = wpool.tile([128, 16], BF16, tag="actw")
            nc.scalar.activation(actw[:], junk[:, 0:16], AF.Prelu, alpha=NEG)
            nc.scalar.activation(actw[:], junk[:, 0:16], AF.Exp)
            nc.scalar.copy(actw[:], junk[:, 0:16])  # Copy shares the exp table set

            # ---- vqpT bias columns: vqp^T[k,b] = sum_d Wq[d,k] vQ[b,d] + bq
            wq16 = pk16_sb[:, 0:1024].rearrange("p (c k) -> p c k", c=DC)
            vqt16 = pk16_sb[:, 1024:1040].rearrange("p (c b) -> p c b", c=DC)
            bq16 = pk16_sb[:, 1040:1042]
            wp16 = pk16_sb[:, 1042:1074].rearrange("p (i j) -> p i j", i=2)
            id16 = pk16_sb[:, 1074:1090]
            nc.vector.tensor_copy(wp8[:], wp16[:])
            bq32 = cpool.tile([128, KC], F32, tag="bq32")
            nc.vector.tensor_copy(bq32[:], bq16[:])
            for kc in range(KC):
                vqpt_ps = psm.tile([128, BLOC], F32, tag="small", name=f"vqps{kc}")
                for c in range(DC):
                    nc.tensor.matmul(
                        vqpt_ps[:],
                        wq16[:, c, kc * 128 : (kc + 1) * 128],
                        vqt16[:, c, :],
                        start=(c == 0),
                        stop=(c == DC - 1),
                    )
                nc.vector.tensor_scalar(
                    vqpt_sb[:, kc, :], vqpt_ps[:], bq32[:, kc : kc + 1], None, ALU.add
                )

            scrows = [None] * BLOC

            def phase_scores(b):
                vit = vit_tiles[b]
                scrow = wpool.tile([1, N], BF16, tag="scrow", name=f"scrow{b}")
                scrows[b] = scrow
                for sp in range(2):
                    scps = [
                        psm.tile([1, SUP], F32, tag="small", name=f"scp{b}_{sp}_{h}")
                        for h in range(2)
                    ]
                    ha = wpool.tile([128, KC, 1024], FP8, tag="ha", name=f"ha{b}{sp}")
                    for kc in range(KC):
                        vp = pmm.tile([128, 1024], F32, tag="vp", name=f"vp{b}{sp}{kc}")
                        for cc in range(2):
                            for h in range(2):
                                nc.tensor.matmul(
                                    vp[:, h * SUP : (h + 1) * SUP],
                                    wi8_sb[:, cc, :, kc * 128 : (kc + 1) * 128],
                                    vit[:, sp, cc, :, h * SUP : (h + 1) * SUP],
                                    perf_mode=mybir.MatmulPerfMode.DoubleRow,
                                    start=(cc == 0),
                                    stop=(cc == 1),
                                )
                        # Wi is host-scaled x16 into fp8 normal range; ACT
                        # de-scales for free: ha = lrelu(vp/16 + vqp)
                        nc.scalar.activation(
                            ha[:, kc, :], vp[:], AF.Prelu,
                            bias=vqpt_sb[:, kc, b : b + 1], scale=1.0 / 16, alpha=NEG,
                        )
                    for h in range(2):
                        nc.tensor.matmul(
                            scps[h][:], wp8[:, :, 0:1],
                            ha[:, :, h * SUP : (h + 1) * SUP],
                            perf_mode=mybir.MatmulPerfMode.DoubleRow,
                            start=True, stop=True,
                        )
                    for h in range(2):
                        n0 = sp * 1024 + h * SUP
                        nc.vector.tensor_copy(scrow[0:1, n0 : n0 + SUP], scps[h][:])

            e_cols = [None] * BLOC

            def phase_echain(b):
                scrow = scrows[b]
                # scores -> [16,128] (small SBUF-SBUF gather) -> PE transpose
                # -> PSUM [128,16], exp reads PSUM. (The xbar dma transpose
                # serializes against every in-flight DMA and stalled the
                # e-chain behind the whole input stream.)
                s16 = wpool.tile([16, 128], BF16, tag="s16", name=f"s16_{b}")
                nc.sync.dma_start(
                    out=s16[:], in_=scrow[0:1, :].rearrange("o (t p) -> o t p", p=128)
                )
                s_ps = psm.tile([128, 16], BF16, tag="small", name=f"sps{b}")
                nc.tensor.transpose(s_ps[:], s16[:], id16[0:16, :])

                # [128, 2, 16]: pair partner at +16B so the DoubleRow
                # lhsT AP satisfies the 16B-step ISA constraint.
                # Wp is host-scaled x8 (fp8 range); exp de-scales for free.
                e_col = wpool.tile([128, 2, 16], FP8, tag="ecol", name=f"ecol{b}")
                e_cols[b] = e_col
                nc.scalar.activation(
                    e_col[:].rearrange("p i j -> p j i")[:, 0:8, :],
                    s_ps[:].rearrange("p (j i) -> p j i", i=2),
                    AF.Exp, scale=1.0 / 8, accum_out=zp_sb[:, b : b + 1],
                )

            def phase_u(b):
                vnat, e_col = vnat_tiles[b], e_cols[b]
                # u = e @ vI on the PE: 8 accumulating fp8 DoubleRow matmuls
                ups = psm.tile([1, D], F32, tag="small", name=f"ups{b}")
                NT = N // 128
                for t in range(0, NT, 2):
                    nc.tensor.matmul(
                        ups[:],
                        e_col[:, :, t // 2 : t // 2 + 1],  # pair stride 16B
                        vnat[:, t : t + 2, :],
                        perf_mode=mybir.MatmulPerfMode.DoubleRow,
                        start=(t == 0),
                        stop=(t == NT - 2),
                    )
                nc.vector.tensor_copy(u_sb[:, b, :], ups[:])

            # Pipeline: all scores phases run back-to-back on the PE with the
            # e-chains (DVE copies / s16 gather / PE transpose / ACT exp)
            # threaded between them; the u matmul phases are deferred to the
            # end so no u phase ever stalls the PE FIFO waiting for its e.
            # The PE transpose of batch b is emitted one scores-phase late so
            # its s16 input is always ready before the PE reaches it.
            phase_scores(0)
            phase_scores(1)
            phase_echain(0)
            phase_scores(2)
            phase_echain(1)
            phase_scores(3)
            phase_echain(2)
            phase_u(0)
            phase_u(1)
            phase_u(2)
            phase_echain(3)
            phase_u(3)

            nc.sync.dma_start(out=u_d[:], in_=u_sb[:])
            nc.sync.dma_start(out=zp_d[:], in_=zp_sb[:])

    nc.compile()
    return nc


_NC = None


def _get_nc():
    global _NC
    if _NC is None:
        _NC = build_nc()
    return _NC


def kernel(vI, vQ, Wi, Wq, bq, Wp, bp, **_unused):
    vI = np.asarray(vI, dtype=np.float32)
    vQ = np.asarray(vQ, dtype=np.float32)
    Wi = np.asarray(Wi, dtype=np.float32)
    Wq = np.asarray(Wq, dtype=np.float32)
    bq = np.asarray(bq, dtype=np.float32)
    Wp = np.asarray(Wp, dtype=np.float32)
    # bp shifts every score equally -> cancels in softmax; ignored (and 0).

    bf = ml_dtypes.bfloat16
    f8 = ml_dtypes.float8_e4m3

    vi8 = vI.astype(f8)
    # vit: [B, p, sp, cc, i, n1024]; d = cc*256 + i*128 + p, n = sp*1024 + n'
    vit = np.ascontiguousarray(
        vi8.reshape(B, 2, 1024, 2, 2, 128).transpose(0, 5, 1, 3, 4, 2)
    )
    # vnat: [B, pn, t, d]; n = t*128 + pn
    vnat = np.ascontiguousarray(
        vi8.reshape(B, 16, 128, D).transpose(0, 2, 1, 3)
    )
    wi8 = np.ascontiguousarray(
        (Wi * 16.0).reshape(2, 2, 128, K).transpose(2, 0, 1, 3)
    ).astype(f8)                                                  # [128,cc,i,K]
    wq_h = Wq.reshape(DC, 128, K).transpose(1, 0, 2).reshape(128, DC * K)
    bq_h = bq.reshape(KC, 128).T                                  # [128,KC]
    wp_pad = np.zeros((128, 2, 16), np.float32)
    wp_pad[:, :, 0] = Wp[:, 0].reshape(KC, 128).T * 8.0
    id16 = np.zeros((128, 16), np.float32)
    id16[0:16, :] = np.eye(16, dtype=np.float32)

    def pk16_for(core):
        vqc = vQ[core * BLOC : (core + 1) * BLOC]                 # [BLOC, D]
        vqt = vqc.T.reshape(DC, 128, BLOC).transpose(1, 0, 2)     # [128,DC,BLOC]
        return np.ascontiguousarray(
            np.concatenate(
                [wq_h, vqt.reshape(128, DC * BLOC), bq_h,
                 wp_pad.reshape(128, 32), id16],
                axis=1,
            ).astype(bf)
        )

    in_maps = []
    for c in range(NCORES):
        in_maps.append(
            {
                "vit": vit[c * BLOC : (c + 1) * BLOC],
                "vnat": vnat[c * BLOC : (c + 1) * BLOC],
                "wi8": wi8,
                "pk16": pk16_for(c),
            }
        )

    nc = _get_nc()
    res = run_bass_kernel_spmd(
        nc, in_maps, list(range(NCORES)),
        trace=bool(int(os.environ.get("KERNEL_TRACE", "0"))),
        tmpdir=globals().get("TRACE_TMPDIR"),
    )
    kernel.last_results = res

    # host-side exact fp32 finish: out = (u @ Wi)/Z + vQp
    vQp = vQ @ Wq + bq
    out = np.empty((B, K), np.float32)
    for c in range(NCORES):
        u = np.asarray(res.results[c]["u"], np.float32)[0]        # [BLOC, D]
        zp = np.asarray(res.results[c]["zp"], np.float32)         # [128, BLOC]
        Z = zp.sum(axis=0)                                        # [BLOC]
        att = (u @ Wi) / Z[:, None]                               # [BLOC, K]
        out[c * BLOC : (c + 1) * BLOC] = att + vQp[c * BLOC : (c + 1) * BLOC]
    return out


# revision 18
# speedup vs baseline: 1.2077x; 1.1310x over previous
"""Trainium2 Bass kernel for the attention-pooling module (v3).

Reference math (B=32, N=2048, D=512, K=256):
    vIp   = vI @ Wi                                   [B,N,K]
    vQp   = vQ @ Wq + bq                              [B,K]
    ha    = leaky_relu(vIp + vQp[:,None,:], 0.01)     [B,N,K]
    scores= ha @ Wp[:,0] + bp                         [B,N]   (bp cancels in softmax)
    pi    = softmax(scores, -1)                       [B,N]
    out   = einsum("bn,bnk->bk", pi, vIp) + vQp       [B,K]

Device (per core, 4 batches, data-parallel over B) computes only what needs
the bulk tensor:
    scores path: vpT = (16*Wi)^T @ vIT   (fp8 DoubleRow matmuls, K on partitions)
                 ha  = ACT Lrelu(vpT/16 + vqpT)  -> fp8
                 scores = (8*Wp)^T @ ha  (PE)  -> exp on [128,16] layout -> e fp8
    attention:   u = e @ vI  (fp8 DoubleRow matmuls vs the natural-layout copy)
                 Z = sum(e)  via the exp's accum_out
Host computes the precision-critical small math exactly in fp32:
    vQp = vQ@Wq + bq,  att = (u @ Wi)/Z,  out = att + vQp.
The device-side wq/vq/bq only shape the softmax weights, so they ride bf16.

Schedule notes:
  - 9 warm-up matmuls on a zeroed tile spin the PE HAM clock-gate up to
    2.4 GHz during the DMA preamble (otherwise the whole kernel runs at
    1.2 GHz in bursts: 39 us of throttled PE time in the v2 trace).
  - DRAM layouts keep each partition's data contiguous (128-descriptor DMAs;
    ~0.6 us HWDGE issue each instead of 1.5 us for 512-descriptor patterns).
  - Per-batch loads are interleaved vit(b+1) before vnat(b) so the PE never
    waits on the next scores tile; u(b) matmuls slot between scores(b+1)
    and scores(b+2).
"""

import os
import sys

sys.path.insert(0, "/opt/trn_rl_repo")

import numpy as np
import ml_dtypes

from concourse import bass, bacc, tile, mybir
from concourse.bass_utils import run_bass_kernel_spmd

dt = mybir.dt
F32, BF16, FP8 = dt.float32, dt.bfloat16, dt.float8e4
AF = mybir.ActivationFunctionType
ALU = mybir.AluOpType

B, N, D, K = 32, 2048, 512, 256
NCORES = 8
BLOC = B // NCORES           # 4 batches per core
SUP = 512                    # scores supertile (PSUM-bank limited)
DC = D // 128                # 4
KC = K // 128                # 2
NEG = 0.01
NWARM = 6                    # ~2.6 us of warm-up matmuls


def build_nc():
    nc = bacc.Bacc("TRN2", target_bir_lowering=False, debug=False)

    # vit: d-on-partitions layout; [b, p, sp, cc, i, n1024], d = cc*256+i*128+p,
    # n = sp*1024 + n1024. Each partition row is 8 KiB contiguous in DRAM.
    vit_d = nc.dram_tensor("vit", [BLOC, 128, 2, 2, 2, 1024], FP8, kind="ExternalInput")
    # vnat: n-on-partitions layout; [b, pn, t, d], n = t*128 + pn.
    vnat_d = nc.dram_tensor("vnat", [BLOC, 128, 16, D], FP8, kind="ExternalInput")
    wi8_d = nc.dram_tensor("wi8", [128, 2, 2, K], FP8, kind="ExternalInput")
    # pk16: wq(1024) | vqt(16) | bq(2) | wp_dr(2x16, wp*8 in col j=0) | id16(16)
    pk16_d = nc.dram_tensor("pk16", [128, 1090], BF16, kind="ExternalInput")
    u_d = nc.dram_tensor("u", [1, BLOC, D], F32, kind="ExternalOutput")
    zp_d = nc.dram_tensor("zp", [128, BLOC], F32, kind="ExternalOutput")

    with tile.TileContext(nc) as tc:
        with (
            tc.tile_pool(name="const", bufs=1) as cpool,
            tc.tile_pool(name="stream", bufs=4) as spool,
            tc.tile_pool(name="work", bufs=3) as wpool,
            tc.tile_pool(name="pmm", bufs=2, space=bass.MemorySpace.PSUM) as pmm,
            tc.tile_pool(name="psm", bufs=2, space=bass.MemorySpace.PSUM) as psm,
            tc.tile_pool(name="patt", bufs=2, space=bass.MemorySpace.PSUM) as patt,
        ):
            pk16_sb = cpool.tile([128, 1090], BF16, tag="pk16")
            wi8_sb = cpool.tile([128, 2, 2, K], FP8, tag="wi8")
            junk = cpool.tile([128, SUP], FP8, tag="junk")
            vqpt_sb = cpool.tile([128, KC, BLOC], F32, tag="vqpt")
            wp8 = cpool.tile([128, 2, 16], FP8, tag="wp8")
            u_sb = cpool.tile([1, BLOC, D], F32, tag="usb")
            zp_sb = cpool.tile([128, BLOC], F32, tag="zpsb")

            vit_tiles = [
                spool.tile([128, 2, 2, 2, 1024], FP8, tag="vit", name=f"vit{b}")
                for b in range(BLOC)
            ]
            vnat_tiles = [
                spool.tile([128, 16, D], FP8, tag="vnat", name=f"vnat{b}")
                for b in range(BLOC)
            ]

            # ---- input DMAs ----------------------------------------------
            # vit0 lands in (sp, cc) quarters so the first matmul can start
            # as soon as ~0.7 MB (pk16+wi8+quarter) has streamed in. All vit
            # (scores-path) tiles stream before any vnat: the scores phases
            # are the PE critical path, the u phases run at the end.
            nc.sync.dma_start(out=pk16_sb[:], in_=pk16_d[:])
            nc.sync.dma_start(out=vit_tiles[0][:, 0, 0], in_=vit_d[0][:, 0, 0])
            nc.sync.dma_start(out=wi8_sb[:], in_=wi8_d[:])
            nc.sync.dma_start(out=vit_tiles[0][:, 0, 1], in_=vit_d[0][:, 0, 1])
            nc.sync.dma_start(out=vit_tiles[0][:, 1, 0], in_=vit_d[0][:, 1, 0])
            nc.sync.dma_start(out=vit_tiles[0][:, 1, 1], in_=vit_d[0][:, 1, 1])
            nc.sync.dma_start(out=vit_tiles[1][:], in_=vit_d[1])
            nc.sync.dma_start(out=vit_tiles[2][:], in_=vit_d[2])
            nc.sync.dma_start(out=vit_tiles[3][:], in_=vit_d[3])
            nc.sync.dma_start(out=vnat_tiles[0][:], in_=vnat_d[0])
            nc.sync.dma_start(out=vnat_tiles[1][:], in_=vnat_d[1])
            nc.sync.dma_start(out=vnat_tiles[2][:], in_=vnat_d[2])
            nc.sync.dma_start(out=vnat_tiles[3][:], in_=vnat_d[3])

            # ---- PE warm-up: HAM un-throttles after ~3.4 us of activity --
            nc.gpsimd.memset(junk[:], 0)
            for w in range(NWARM):
                wps = pmm.tile([128, SUP], F32, tag="vp", name=f"warm{w}")
                nc.tensor.matmul(
                    wps[:], junk[:, 0:128], junk[:], start=True, stop=True
                )
            # preload the ACT tables (Lrelu/Exp/Copy) while the DMAs stream;
            # a lazy mid-kernel ACT_TABLE_LOAD costs 1.28us on the e-chain
            actw = wpool.tile([128, 16], BF16, tag="actw")
            nc.scalar.activation(actw[:], junk[:, 0:16], AF.Prelu, alpha=NEG)
            nc.scalar.activation(actw[:], junk[:, 0:16], AF.Exp)
            nc.scalar.copy(actw[:], junk[:, 0:16])  # Copy shares the exp table set

            # ---- vqpT bias columns: vqp^T[k,b] = sum_d Wq[d,k] vQ[b,d] + bq
            wq16 = pk16_sb[:, 0:1024].rearrange("p (c k) -> p c k", c=DC)
            vqt16 = pk16_sb[:, 1024:1040].rearrange("p (c b) -> p c b", c=DC)
            bq16 = pk16_sb[:, 1040:1042]
            wp16 = pk16_sb[:, 1042:1074].rearrange("p (i j) -> p i j", i=2)
            id16 = pk16_sb[:, 1074:1090]
            nc.vector.tensor_copy(wp8[:], wp16[:])
            bq32 = cpool.tile([128, KC], F32, tag="bq32")
            nc.vector.tensor_copy(bq32[:], bq16[:])
            for kc in range(KC):
                vqpt_ps = psm.tile([128, BLOC], F32, tag="small", name=f"vqps{kc}")
                for c in range(DC):
                    nc.tensor.matmul(
                        vqpt_ps[:],
                        wq16[:, c, kc * 128 : (kc + 1) * 128],
                        vqt16[:, c, :],
                        start=(c == 0),
                        stop=(c == DC - 1),
                    )
                nc.vector.tensor_scalar(
                    vqpt_sb[:, kc, :], vqpt_ps[:], bq32[:, kc : kc + 1], None, ALU.add
                )

            scrows = [None] * BLOC

            def phase_scores(b):
                vit = vit_tiles[b]
                scrow = wpool.tile([1, N], BF16, tag="scrow", name=f"scrow{b}")
                scrows[b] = scrow
                for sp in range(2):
                    scps = [
                        psm.tile([1, SUP], F32, tag="small", name=f"scp{b}_{sp}_{h}")
                        for h in range(2)
                    ]
                    ha = wpool.tile([128, KC, 1024], FP8, tag="ha", name=f"ha{b}{sp}")
                    for kc in range(KC):
                        vp = pmm.tile([128, 1024], F32, tag="vp", name=f"vp{b}{sp}{kc}")
                        for cc in range(2):
                            for h in range(2):
                                nc.tensor.matmul(
                                    vp[:, h * SUP : (h + 1) * SUP],
                                    wi8_sb[:, cc, :, kc * 128 : (kc + 1) * 128],
                                    vit[:, sp, cc, :, h * SUP : (h + 1) * SUP],
                                    perf_mode=mybir.MatmulPerfMode.DoubleRow,
                                    start=(cc == 0),
                                    stop=(cc == 1),
                                )
                        # Wi is host-scaled x16 into fp8 normal range; ACT
                        # de-scales for free: ha = lrelu(vp/16 + vqp)
                        nc.scalar.activation(
                            ha[:, kc, :], vp[:], AF.Prelu,
                            bias=vqpt_sb[:, kc, b : b + 1], scale=1.0 / 16, alpha=NEG,
                        )
                    for h in range(2):
                        nc.tensor.matmul(
                            scps[h][:], wp8[:, :, 0:1],
                            ha[:, :, h * SUP : (h + 1) * SUP],
                            perf_mode=mybir.MatmulPerfMode.DoubleRow,
                            start=True, stop=True,
                        )
                    for h in range(2):
                        n0 = sp * 1024 + h * SUP
                        nc.vector.tensor_copy(scrow[0:1, n0 : n0 + SUP], scps[h][:])

            e_cols = [None] * BLOC

            def phase_echain(b):
                scrow = scrows[b]
                # scores -> [16,128] (small SBUF-SBUF gather) -> PE transpose
                # -> PSUM [128,16], exp reads PSUM. (The xbar dma transpose
                # serializes against every in-flight DMA and stalled the
                # e-chain behind the whole input stream.)
                s16 = wpool.tile([16, 128], BF16, tag="s16", name=f"s16_{b}")
                nc.sync.dma_start(
                    out=s16[:], in_=scrow[0:1, :].rearrange("o (t p) -> o t p", p=128)
                )
                s_ps = patt.tile([128, 16], BF16, tag="att", name=f"sps{b}")
                nc.tensor.transpose(s_ps[:], s16[:], id16[0:16, :])

                # [128, 2, 16]: pair partner at +16B so the DoubleRow
                # lhsT AP satisfies the 16B-step ISA constraint.
                # Wp is host-scaled x8 (fp8 range); exp de-scales for free.
                e_col = wpool.tile([128, 2, 16], FP8, tag="ecol", name=f"ecol{b}")
                e_cols[b] = e_col
                nc.scalar.activation(
                    e_col[:].rearrange("p i j -> p j i")[:, 0:8, :],
                    s_ps[:].rearrange("p (j i) -> p j i", i=2),
                    AF.Exp, scale=1.0 / 8, accum_out=zp_sb[:, b : b + 1],
                )

            def phase_u(b):
                vnat, e_col = vnat_tiles[b], e_cols[b]
                # u = e @ vI on the PE: 8 accumulating fp8 DoubleRow matmuls
                ups = patt.tile([1, D], F32, tag="att", name=f"ups{b}")
                NT = N // 128
                for t in range(0, NT, 2):
                    nc.tensor.matmul(
                        ups[:],
                        e_col[:, :, t // 2 : t // 2 + 1],  # pair stride 16B
                        vnat[:, t : t + 2, :],
                        perf_mode=mybir.MatmulPerfMode.DoubleRow,
                        start=(t == 0),
                        stop=(t == NT - 2),
                    )
                nc.vector.tensor_copy(u_sb[:, b, :], ups[:])

            # Pipeline: all scores phases run back-to-back on the PE with the
            # e-chains (DVE copies / s16 gather / PE transpose / ACT exp)
            # threaded between them; the u matmul phases are deferred to the
            # end so no u phase ever stalls the PE FIFO waiting for its e.
            # The PE transpose of batch b is emitted one scores-phase late so
            # its s16 input is always ready before the PE reaches it.
            phase_scores(0)
            phase_scores(1)
            phase_echain(0)
            phase_scores(2)
            phase_echain(1)
            phase_scores(3)
            phase_echain(2)
            phase_u(0)
            phase_u(1)
            phase_u(2)
            phase_echain(3)
            phase_u(3)

            nc.sync.dma_start(out=u_d[:], in_=u_sb[:])
            nc.sync.dma_start(out=zp_d[:], in_=zp_sb[:])

    nc.compile()
    return nc


_NC = None


def _get_nc():
    global _NC
    if _NC is None:
        _NC = build_nc()
    return _NC


def kernel(vI, vQ, Wi, Wq, bq, Wp, bp, **_unused):
    vI = np.asarray(vI, dtype=np.float32)
    vQ = np.asarray(vQ, dtype=np.float32)
    Wi = np.asarray(Wi, dtype=np.float32)
    Wq = np.asarray(Wq, dtype=np.float32)
    bq = np.asarray(bq, dtype=np.float32)
    Wp = np.asarray(Wp, dtype=np.float32)
    # bp shifts every score equally -> cancels in softmax; ignored (and 0).

    bf = ml_dtypes.bfloat16
    f8 = ml_dtypes.float8_e4m3

    vi8 = vI.astype(f8)
    # vit: [B, p, sp, cc, i, n1024]; d = cc*256 + i*128 + p, n = sp*1024 + n'
    vit = np.ascontiguousarray(
        vi8.reshape(B, 2, 1024, 2, 2, 128).transpose(0, 5, 1, 3, 4, 2)
    )
    # vnat: [B, pn, t, d]; n = t*128 + pn
    vnat = np.ascontiguousarray(
        vi8.reshape(B, 16, 128, D).transpose(0, 2, 1, 3)
    )
    wi8 = np.ascontiguousarray(
        (Wi * 16.0).reshape(2, 2, 128, K).transpose(2, 0, 1, 3)
    ).astype(f8)                                                  # [128,cc,i,K]
    wq_h = Wq.reshape(DC, 128, K).transpose(1, 0, 2).reshape(128, DC * K)
    bq_h = bq.reshape(KC, 128).T                                  # [128,KC]
    wp_pad = np.zeros((128, 2, 16), np.float32)
    wp_pad[:, :, 0] = Wp[:, 0].reshape(KC, 128).T * 8.0
    id16 = np.zeros((128, 16), np.float32)
    id16[0:16, :] = np.eye(16, dtype=np.float32)

    def pk16_for(core):
        vqc = vQ[core * BLOC : (core + 1) * BLOC]                 # [BLOC, D]
        vqt = vqc.T.reshape(DC, 128, BLOC).transpose(1, 0, 2)     # [128,DC,BLOC]
        return np.ascontiguousarray(
            np.concatenate(
                [wq_h, vqt.reshape(128, DC * BLOC), bq_h,
                 wp_pad.reshape(128, 32), id16],
                axis=1,
            ).astype(bf)
        )

    in_maps = []
    for c in range(NCORES):
        in_maps.append(
            {
                "vit": vit[c * BLOC : (c + 1) * BLOC],
                "vnat": vnat[c * BLOC : (c + 1) * BLOC],
                "wi8": wi8,
                "pk16": pk16_for(c),
            }
        )

    nc = _get_nc()
    res = run_bass_kernel_spmd(
        nc, in_maps, list(range(NCORES)),
        trace=bool(int(os.environ.get("KERNEL_TRACE", "0"))),
        tmpdir=globals().get("TRACE_TMPDIR"),
    )
    kernel.last_results = res

    # host-side exact fp32 finish: out = (u @ Wi)/Z + vQp
    vQp = vQ @ Wq + bq
    out = np.empty((B, K), np.float32)
    for c in range(NCORES):
        u = np.asarray(res.results[c]["u"], np.float32)[0]        # [BLOC, D]
        zp = np.asarray(res.results[c]["zp"], np.float32)         # [128, BLOC]
        Z = zp.sum(axis=0)                                        # [BLOC]
        att = (u @ Wi) / Z[:, None]                               # [BLOC, K]
        out[c * BLOC : (c + 1) * BLOC] = att + vQp[c * BLOC : (c + 1) * BLOC]
    return out
